# revision 31
# baseline (speedup 1.0000x reference)
"""PiLoraLayer TRN2 kernel: y = x + (alpha/r) * sin((2/pi) * (x @ A) @ B).

x: [4, 4096, 4096] f32; A = A_int8 * scale_A (per-col), B = B_int8 * scale_B
(per-col); rank 16 bottleneck.  alpha/r = 2.

v2 strategy (data-parallel over 8 NeuronCores, transposed fp16 streaming):
- Host: shard x's 16384 token rows into 8 shards; stage xh = (x/2) as fp16,
  TRANSPOSED to [4096, 2048] per core.  This kills the on-device PE
  transpose pass and its PSUM->SBUF copy pass of v1 (~210us PE + ~100us
  ACT), and fp16 halves DMA traffic both ways.  fp16 x costs ~3e-3 rel err
  (budget 2e-2); bf16 would cost ~5e-2.
- Weights: A_q as exact fp16 ints; Bp = 2 * scale_A[:,None] * B_q *
  scale_B[None,:] / pi^2 (f32), so u := (xh @ A) @ Bp = arg/(2*pi) with
  arg = (2/pi)*h the true sin argument.
- Device per core (xT fully SBUF-resident, 128 KB/partition):
  - mm1: h1[16, tok] = sum_k A_k.T @ xh[k-chunk, tok]  (fp16, PSUM acc)
  - mm2 (transposed out): u[128, tok] per H-chunk = Bp_c.T @ h1  (f32r)
  - range reduction (Sin LUT domain is [-pi, pi]): k = (u+1.5*2^23)-1.5*2^23
    in one DVE tensor_scalar (PSUM-read, bf16 out, exact for |k|<=256);
    PE accumulates -k via bf16 negative-identity matmul -> frac in [-.5,.5]
  - ACT: s = sin(2*pi*frac) -> fp16, 1024-wide calls (2 H-chunks paired)
  - residual: yh = xh + s in place (plain fp16 tensor_tensor add; the *2 is
    folded out host-side).  Adds alternate DVE/GPSIMD to balance: DVE also
    carries the krounds (PSUM-read, 1x mode, ~78us), GPSIMD does nothing
    else but is ~4x slower per element, so a 50/50 pair split lands both
    near ~95-100us.
  - DMA yh out as fp16 (sync/HWDGE).
- Host: y = 2 * float32(yh).T  (exact exponent shift).
"""

import sys

sys.path.insert(0, "/opt/trn_rl_repo")

import numpy as np

import concourse.bacc as bacc
import concourse.tile as tile
from concourse import mybir
from concourse.bass_utils import run_bass_kernel_spmd

P = 128
HIDDEN = 4096
RANK = 16
KC = HIDDEN // P  # 32 hidden chunks
N_CORES = 8
TOTAL_ROWS = 4 * 4096
ROWS = TOTAL_ROWS // N_CORES  # 2048 tokens per core
TB = 512  # steady-state token block
GRP = 2  # H-chunks per kround/sin/add group (PSUM: 3 live groups * GRP banks)
MAGIC = 12582912.0  # 1.5 * 2^23: f32 add/sub rounds to nearest integer
SCALE_2PI = 6.283185  # slightly < 2*pi so the LUT arg stays inside [-pi, pi]

F32 = mybir.dt.float32
F32R = mybir.dt.float32r
BF16 = mybir.dt.bfloat16
FP16 = mybir.dt.float16

# fraction of residual-add groups that run on DVE (rest on GPSIMD).  DVE also
# owns the kround pass (~81us); GPSIMD only does adds but is slower per
# element, so a ~1/4 : 3/4 split lands both near ~105us.
DVE_ADD_NUM = 1
DVE_ADD_DEN = 4


def build_nc(rows: int = ROWS):
    """Per-core Bass program for a transposed [4096, rows] fp16 token shard."""
    nc = bacc.Bacc(
        "TRN2",
        target_bir_lowering=False,
        debug=False,
        enable_asserts=False,
        num_devices=N_CORES,
    )
    x_d = nc.dram_tensor("x", [HIDDEN, rows], FP16, kind="ExternalInput").ap()
    a_d = nc.dram_tensor("A", [HIDDEN, RANK], FP16, kind="ExternalInput").ap()
    bp_d = nc.dram_tensor("Bp", [RANK, HIDDEN], FP16, kind="ExternalInput").ap()
    y_d = nc.dram_tensor("out", [HIDDEN, rows], BF16, kind="ExternalOutput").ap()

    x_r = x_d.rearrange("(k p) t -> p k t", p=P)
    y_r = y_d.rearrange("(k p) t -> p k t", p=P)

    with tile.TileContext(nc) as tc:
        with (
            tc.tile_pool(name="singles", bufs=1) as singles,
            tc.tile_pool(name="h1sb", bufs=2) as h1sb_pool,
            tc.tile_pool(name="kqp", bufs=4) as kq_pool,
            tc.tile_pool(name="sp", bufs=5) as s_pool,
            tc.tile_pool(name="yp", bufs=3) as y_pool,
            tc.tile_pool(name="h1p", bufs=2, space="PSUM") as h1_psum,
            tc.tile_pool(name="up", bufs=3, space="PSUM") as u_psum,
        ):
            nident_bf = singles.tile([P, P], BF16)
            nc.gpsimd.memset(nident_bf[:], 0.0)
            nc.gpsimd.affine_select(
                out=nident_bf[:],
                in_=nident_bf[:],
                compare_op=mybir.AluOpType.not_equal,
                fill=-1.0,
                base=0,
                pattern=[[-1, P]],
                channel_multiplier=1,
            )
            a_sb = singles.tile([P, KC, RANK], FP16)
            nc.sync.dma_start(
                out=a_sb[:], in_=a_d.rearrange("(k p) r -> p k r", p=P)
            )
            bp_sb = singles.tile([RANK, HIDDEN], FP16)
            nc.sync.dma_start(out=bp_sb[:], in_=bp_d[:, :])

            # resident x (fp16): 128 KB/partition for rows=2048
            xs = singles.tile([P, KC, rows], FP16)

            # token-block layout; small edge blocks halve pipeline fill/drain
            layout = []
            r = 0
            if rows <= TB:
                sizes = [rows]
            else:
                sizes = [TB // 2] + [TB] * ((rows - TB) // TB) + [TB // 2]
            for tok in sizes:
                layout.append((r, tok))
                r += tok
            assert r == rows

            # stage all input DMAs up front; mm1 of block b waits only on its
            # own slice via tile dependency tracking.  All descriptor
            # generation (~2.2ns/desc on the issuing engine) stays on sync,
            # whose out-DMAs only start after the up-front in-gen anyway.
            # The first block is k-halved so its mm1 can start sooner.
            for bi, (row0, tok) in enumerate(layout):
                if bi == 0:
                    for kh in range(2):
                        ks = slice(kh * (KC // 2), (kh + 1) * (KC // 2))
                        nc.sync.dma_start(
                            out=xs[:, ks, row0 : row0 + tok],
                            in_=x_r[:, ks, row0 : row0 + tok],
                        )
                else:
                    nc.sync.dma_start(
                        out=xs[:, :, row0 : row0 + tok],
                        in_=x_r[:, :, row0 : row0 + tok],
                    )

            adds = 0
            y4 = None  # output tile shared by 2 consecutive groups

            def finish_group(st):
                """-k accumulate, sin, residual add; DMA out every 2nd group."""
                nonlocal adds, y4
                u, kq, c0, g, row0, tok = st
                for j in range(g):
                    nc.tensor.matmul(
                        u[:, j, :tok],
                        nident_bf[:],
                        kq[:, j, :],
                        start=False,
                        stop=True,
                        skip_group_check=True,
                    )
                s = s_pool.tile([P, g, tok], FP16)
                nc.scalar.activation(
                    out=s[:],
                    in_=u[:, :g, :tok],
                    func=mybir.ActivationFunctionType.Sin,
                    scale=SCALE_2PI,
                )
                half = (c0 // GRP) % 2
                if half == 0:
                    y4 = y_pool.tile([P, 2 * GRP, tok], BF16)
                eng = (
                    nc.vector if (adds % DVE_ADD_DEN) < DVE_ADD_NUM else nc.gpsimd
                )
                adds += 1
                eng.tensor_tensor(
                    y4[:, half * GRP : half * GRP + g, :],
                    s[:],
                    xs[:, c0 : c0 + g, row0 : row0 + tok],
                    mybir.AluOpType.add,
                )
                if half == 1:
                    d0 = c0 - GRP
                    nc.sync.dma_start(
                        out=y_r[:, d0 : d0 + 2 * GRP, row0 : row0 + tok],
                        in_=y4[:],
                    )

            def mm1_chunk(row0, tok, h1_ps, k):
                nc.tensor.matmul(
                    h1_ps[:],
                    a_sb[:, k, :],
                    xs[:, k, row0 : row0 + tok],
                    start=(k == 0),
                    stop=(k == KC - 1),
                )

            # up-to-2-deep pending queue: build two groups, then finish two.
            # The PE stream becomes runs of 4 same-shaped stationaries
            # ([mm2 x4][-k x4]) instead of alternating every 2 matmuls;
            # LDWEIGHTS hides within a run, alternation costs ~320ns/matmul.
            pend = []
            NG = KC // GRP  # groups per block

            for row0, tok in layout:
                h1_ps = h1_psum.tile([RANK, tok], F32)
                for k in range(KC):
                    mm1_chunk(row0, tok, h1_ps, k)
                h1_sb = h1sb_pool.tile([RANK, tok], FP16)
                nc.vector.tensor_copy(h1_sb[:], h1_ps[:])

                for gi in range(NG):
                    c0 = gi * GRP
                    # one PSUM bank (512 f32) per H-chunk: accumulation
                    # groups are bank-granular, so two chunks must never
                    # share a bank (start=True would clear the sibling's
                    # has_written and break the -k accumulate)
                    u = u_psum.tile([P, GRP, max(tok, 512)], F32)
                    for j in range(GRP):
                        c = c0 + j
                        nc.tensor.matmul(
                            u[:, j, :tok],
                            bp_sb[:, c * P : (c + 1) * P],
                            h1_sb[:],
                            start=True,
                            stop=True,
                        )
                    kq = kq_pool.tile([P, GRP, tok], BF16)
                    nc.vector.tensor_scalar(
                        kq[:],
                        u[:, :, :tok],
                        MAGIC,
                        MAGIC,
                        mybir.AluOpType.add,
                        mybir.AluOpType.subtract,
                    )
                    if len(pend) == 2:
                        finish_group(pend.pop(0))
                        finish_group(pend.pop(0))
                    pend.append((u, kq, c0, GRP, row0, tok))

            for st in pend:
                finish_group(st)

    nc.compile()
    return nc


_NC_CACHE: dict[int, object] = {}


def _get_nc(rows: int = ROWS):
    nc = _NC_CACHE.get(rows)
    if nc is None:
        nc = build_nc(rows)
        _NC_CACHE[rows] = nc
    return nc


def _prep_weights(A_int8, B_int8, scale_A, scale_B):
    # A/1024 is exact in fp16 (|A|<=127 ints, exponent shift); the
    # compensating *1024 on Bp lifts its tiny entries into fp16's normal
    # range (min |Bp| ~2e-4 > 6.1e-5).  fp16 mm2 streams 1 cyc/col on PE
    # where f32r ran 4-pass.
    a16 = np.ascontiguousarray(
        (A_int8.astype(np.float32) * np.float32(1.0 / 1024.0)).astype(np.float16)
    )
    bp = np.ascontiguousarray(
        (
            scale_A.astype(np.float32)[:, None]
            * B_int8.astype(np.float32)
            * scale_B.astype(np.float32)[None, :]
            # 2.0: compensates x/2 staging; 1024: compensates A/1024
            * np.float32(2048.0 / (np.pi * np.pi))
        ).astype(np.float16)
    )
    return a16, bp


def _shard_inputs(x, A_int8, B_int8, scale_A, scale_B):
    xf = x.reshape(TOTAL_ROWS, HIDDEN)
    xh = (xf.astype(np.float32) * np.float32(0.5)).astype(np.float16)
    a16, bp = _prep_weights(A_int8, B_int8, scale_A, scale_B)
    in_maps = []
    for i in range(N_CORES):
        xt = np.ascontiguousarray(xh[i * ROWS : (i + 1) * ROWS].T)
        in_maps.append({"x": xt, "A": a16, "Bp": bp})
    return in_maps


def _gather_output(res, orig_shape):
    y = np.empty((TOTAL_ROWS, HIDDEN), dtype=np.float32)
    for i in range(N_CORES):
        # device computed yh = x/2 + sin(...); y = 2*yh (exact x2 in f32)
        y[i * ROWS : (i + 1) * ROWS] = res.results[i]["out"].T
    y *= np.float32(2.0)
    return y.reshape(orig_shape)


def kernel(x, A_int8, B_int8, scale_A, scale_B):
    x = np.asarray(x)
    orig_shape = x.shape
    in_maps = _shard_inputs(
        x,
        np.asarray(A_int8),
        np.asarray(B_int8),
        np.asarray(scale_A),
        np.asarray(scale_B),
    )
    nc = _get_nc(ROWS)
    res = run_bass_kernel_spmd(nc, in_maps, core_ids=list(range(N_CORES)))
    return _gather_output(res, orig_shape)


# revision 33
# speedup vs baseline: 1.2026x; 1.2026x over previous
"""PiLoraLayer TRN2 kernel: y = x + (alpha/r) * sin((2/pi) * (x @ A) @ B).

x: [4, 4096, 4096] f32; A = A_int8 * scale_A (per-col), B = B_int8 * scale_B
(per-col); rank 16 bottleneck.  alpha/r = 2.

v2 strategy (data-parallel over 8 NeuronCores, transposed fp16 streaming):
- Host: shard x's 16384 token rows into 8 shards; stage xh = (x/2) as fp16,
  TRANSPOSED to [4096, 2048] per core.  This kills the on-device PE
  transpose pass and its PSUM->SBUF copy pass of v1 (~210us PE + ~100us
  ACT), and fp16 halves DMA traffic both ways.  fp16 x costs ~3e-3 rel err
  (budget 2e-2); bf16 would cost ~5e-2.
- Weights: A_q as exact fp16 ints; Bp = 2 * scale_A[:,None] * B_q *
  scale_B[None,:] / pi^2 (f32), so u := (xh @ A) @ Bp = arg/(2*pi) with
  arg = (2/pi)*h the true sin argument.
- Device per core (xT fully SBUF-resident, 128 KB/partition):
  - mm1: h1[16, tok] = sum_k A_k.T @ xh[k-chunk, tok]  (fp16, PSUM acc)
  - mm2 (transposed out): u[128, tok] per H-chunk = Bp_c.T @ h1  (f32r)
  - range reduction (Sin LUT domain is [-pi, pi]): k = (u+1.5*2^23)-1.5*2^23
    in one DVE tensor_scalar (PSUM-read, bf16 out, exact for |k|<=256);
    PE accumulates -k via bf16 negative-identity matmul -> frac in [-.5,.5]
  - ACT: s = sin(2*pi*frac) -> fp16, 1024-wide calls (2 H-chunks paired)
  - residual: yh = xh + s in place (plain fp16 tensor_tensor add; the *2 is
    folded out host-side).  Adds alternate DVE/GPSIMD to balance: DVE also
    carries the krounds (PSUM-read, 1x mode, ~78us), GPSIMD does nothing
    else but is ~4x slower per element, so a 50/50 pair split lands both
    near ~95-100us.
  - DMA yh out as fp16 (sync/HWDGE).
- Host: y = 2 * float32(yh).T  (exact exponent shift).
"""

import sys

sys.path.insert(0, "/opt/trn_rl_repo")

import numpy as np

import concourse.bacc as bacc
import concourse.tile as tile
from concourse import mybir
from concourse.bass_utils import run_bass_kernel_spmd

P = 128
HIDDEN = 4096
RANK = 16
KC = HIDDEN // P  # 32 hidden chunks
N_CORES = 8
TOTAL_ROWS = 4 * 4096
ROWS = TOTAL_ROWS // N_CORES  # 2048 tokens per core
TB = 512  # steady-state token block
GRP = 2  # H-chunks per kround/sin/add group (PSUM: 3 live groups * GRP banks)
MAGIC = 12582912.0  # 1.5 * 2^23: f32 add/sub rounds to nearest integer
SCALE_2PI = 6.283185  # slightly < 2*pi so the LUT arg stays inside [-pi, pi]

F32 = mybir.dt.float32
F32R = mybir.dt.float32r
BF16 = mybir.dt.bfloat16
FP16 = mybir.dt.float16

# fraction of residual-add groups that run on DVE (rest on GPSIMD).  DVE also
# owns the kround pass (~81us); GPSIMD only does adds but is slower per
# element, so a ~1/4 : 3/4 split lands both near ~105us.
DVE_ADD_NUM = 1
DVE_ADD_DEN = 4
PEND_DEPTH = 1  # built groups queued before finishing (PE run-length knob)


def build_nc(rows: int = ROWS):
    """Per-core Bass program for a transposed [4096, rows] fp16 token shard."""
    nc = bacc.Bacc(
        "TRN2",
        target_bir_lowering=False,
        debug=False,
        enable_asserts=False,
        num_devices=N_CORES,
    )
    x_d = nc.dram_tensor("x", [HIDDEN, rows], FP16, kind="ExternalInput").ap()
    a_d = nc.dram_tensor("A", [HIDDEN, RANK], FP16, kind="ExternalInput").ap()
    bp_d = nc.dram_tensor("Bp", [RANK, HIDDEN], FP16, kind="ExternalInput").ap()
    y_d = nc.dram_tensor("out", [HIDDEN, rows], BF16, kind="ExternalOutput").ap()

    x_r = x_d.rearrange("(k p) t -> p k t", p=P)
    y_r = y_d.rearrange("(k p) t -> p k t", p=P)

    with tile.TileContext(nc) as tc:
        with (
            tc.tile_pool(name="singles", bufs=1) as singles,
            tc.tile_pool(name="h1sb", bufs=2) as h1sb_pool,
            tc.tile_pool(name="kqp", bufs=4) as kq_pool,
            tc.tile_pool(name="sp", bufs=5) as s_pool,
            tc.tile_pool(name="yp", bufs=3) as y_pool,
            tc.tile_pool(name="h1p", bufs=2, space="PSUM") as h1_psum,
            tc.tile_pool(name="up", bufs=3, space="PSUM") as u_psum,
        ):
            nident_bf = singles.tile([P, P], BF16)
            nc.gpsimd.memset(nident_bf[:], 0.0)
            nc.gpsimd.affine_select(
                out=nident_bf[:],
                in_=nident_bf[:],
                compare_op=mybir.AluOpType.not_equal,
                fill=-1.0,
                base=0,
                pattern=[[-1, P]],
                channel_multiplier=1,
            )
            a_sb = singles.tile([P, KC, RANK], FP16)
            nc.sync.dma_start(
                out=a_sb[:], in_=a_d.rearrange("(k p) r -> p k r", p=P)
            )
            bp_sb = singles.tile([RANK, HIDDEN], FP16)
            nc.sync.dma_start(out=bp_sb[:], in_=bp_d[:, :])

            # resident x (fp16): 128 KB/partition for rows=2048
            xs = singles.tile([P, KC, rows], FP16)

            # token-block layout; small edge blocks halve pipeline fill/drain
            layout = []
            r = 0
            if rows <= TB:
                sizes = [rows]
            else:
                sizes = [TB // 2] + [TB] * ((rows - TB) // TB) + [TB // 2]
            for tok in sizes:
                layout.append((r, tok))
                r += tok
            assert r == rows

            # stage all input DMAs up front; mm1 of block b waits only on its
            # own slice via tile dependency tracking.  All descriptor
            # generation (~2.2ns/desc on the issuing engine) stays on sync,
            # whose out-DMAs only start after the up-front in-gen anyway.
            # The first block is k-halved so its mm1 can start sooner.
            for bi, (row0, tok) in enumerate(layout):
                if bi == 0:
                    for kh in range(2):
                        ks = slice(kh * (KC // 2), (kh + 1) * (KC // 2))
                        nc.sync.dma_start(
                            out=xs[:, ks, row0 : row0 + tok],
                            in_=x_r[:, ks, row0 : row0 + tok],
                        )
                else:
                    nc.sync.dma_start(
                        out=xs[:, :, row0 : row0 + tok],
                        in_=x_r[:, :, row0 : row0 + tok],
                    )

            adds = 0
            y4 = None  # output tile shared by 2 consecutive groups

            def finish_group(st):
                """-k accumulate, sin, residual add; DMA out every 2nd group."""
                nonlocal adds, y4
                u, kq, c0, g, row0, tok = st
                for j in range(g):
                    nc.tensor.matmul(
                        u[:, j, :tok],
                        nident_bf[:],
                        kq[:, j, :],
                        start=False,
                        stop=True,
                        skip_group_check=True,
                    )
                s = s_pool.tile([P, g, tok], FP16)
                nc.scalar.activation(
                    out=s[:],
                    in_=u[:, :g, :tok],
                    func=mybir.ActivationFunctionType.Sin,
                    scale=SCALE_2PI,
                )
                half = (c0 // GRP) % 2
                if half == 0:
                    y4 = y_pool.tile([P, 2 * GRP, tok], BF16)
                eng = (
                    nc.vector if (adds % DVE_ADD_DEN) < DVE_ADD_NUM else nc.gpsimd
                )
                adds += 1
                eng.tensor_tensor(
                    y4[:, half * GRP : half * GRP + g, :],
                    s[:],
                    xs[:, c0 : c0 + g, row0 : row0 + tok],
                    mybir.AluOpType.add,
                )
                if half == 1:
                    d0 = c0 - GRP
                    nc.sync.dma_start(
                        out=y_r[:, d0 : d0 + 2 * GRP, row0 : row0 + tok],
                        in_=y4[:],
                    )

            def mm1_chunk(row0, tok, h1_ps, k):
                nc.tensor.matmul(
                    h1_ps[:],
                    a_sb[:, k, :],
                    xs[:, k, row0 : row0 + tok],
                    start=(k == 0),
                    stop=(k == KC - 1),
                )

            # up-to-2-deep pending queue: build two groups, then finish two.
            # The PE stream becomes runs of 4 same-shaped stationaries
            # ([mm2 x4][-k x4]) instead of alternating every 2 matmuls;
            # LDWEIGHTS hides within a run, alternation costs ~320ns/matmul.
            pend = []
            NG = KC // GRP  # groups per block

            for row0, tok in layout:
                h1_ps = h1_psum.tile([RANK, tok], F32)
                for k in range(KC):
                    mm1_chunk(row0, tok, h1_ps, k)
                h1_sb = h1sb_pool.tile([RANK, tok], FP16)
                nc.vector.tensor_copy(h1_sb[:], h1_ps[:])

                for gi in range(NG):
                    c0 = gi * GRP
                    # one PSUM bank (512 f32) per H-chunk: accumulation
                    # groups are bank-granular, so two chunks must never
                    # share a bank (start=True would clear the sibling's
                    # has_written and break the -k accumulate)
                    u = u_psum.tile([P, GRP, max(tok, 512)], F32)
                    for j in range(GRP):
                        c = c0 + j
                        nc.tensor.matmul(
                            u[:, j, :tok],
                            bp_sb[:, c * P : (c + 1) * P],
                            h1_sb[:],
                            start=True,
                            stop=True,
                        )
                    kq = kq_pool.tile([P, GRP, tok], BF16)
                    nc.vector.tensor_scalar(
                        kq[:],
                        u[:, :, :tok],
                        MAGIC,
                        MAGIC,
                        mybir.AluOpType.add,
                        mybir.AluOpType.subtract,
                    )
                    if len(pend) == PEND_DEPTH:
                        for _ in range(PEND_DEPTH):
                            finish_group(pend.pop(0))
                    pend.append((u, kq, c0, GRP, row0, tok))

            for st in pend:
                finish_group(st)

    nc.compile()
    return nc


_NC_CACHE: dict[int, object] = {}


def _get_nc(rows: int = ROWS):
    nc = _NC_CACHE.get(rows)
    if nc is None:
        nc = build_nc(rows)
        _NC_CACHE[rows] = nc
    return nc


def _prep_weights(A_int8, B_int8, scale_A, scale_B):
    # A/1024 is exact in fp16 (|A|<=127 ints, exponent shift); the
    # compensating *1024 on Bp lifts its tiny entries into fp16's normal
    # range (min |Bp| ~2e-4 > 6.1e-5).  fp16 mm2 streams 1 cyc/col on PE
    # where f32r ran 4-pass.
    a16 = np.ascontiguousarray(
        (A_int8.astype(np.float32) * np.float32(1.0 / 1024.0)).astype(np.float16)
    )
    bp = np.ascontiguousarray(
        (
            scale_A.astype(np.float32)[:, None]
            * B_int8.astype(np.float32)
            * scale_B.astype(np.float32)[None, :]
            # 2.0: compensates x/2 staging; 1024: compensates A/1024
            * np.float32(2048.0 / (np.pi * np.pi))
        ).astype(np.float16)
    )
    return a16, bp


def _shard_inputs(x, A_int8, B_int8, scale_A, scale_B):
    xf = x.reshape(TOTAL_ROWS, HIDDEN)
    xh = (xf.astype(np.float32) * np.float32(0.5)).astype(np.float16)
    a16, bp = _prep_weights(A_int8, B_int8, scale_A, scale_B)
    in_maps = []
    for i in range(N_CORES):
        xt = np.ascontiguousarray(xh[i * ROWS : (i + 1) * ROWS].T)
        in_maps.append({"x": xt, "A": a16, "Bp": bp})
    return in_maps


def _gather_output(res, orig_shape):
    y = np.empty((TOTAL_ROWS, HIDDEN), dtype=np.float32)
    for i in range(N_CORES):
        # device computed yh = x/2 + sin(...); y = 2*yh (exact x2 in f32)
        y[i * ROWS : (i + 1) * ROWS] = res.results[i]["out"].T
    y *= np.float32(2.0)
    return y.reshape(orig_shape)


def kernel(x, A_int8, B_int8, scale_A, scale_B):
    x = np.asarray(x)
    orig_shape = x.shape
    in_maps = _shard_inputs(
        x,
        np.asarray(A_int8),
        np.asarray(B_int8),
        np.asarray(scale_A),
        np.asarray(scale_B),
    )
    nc = _get_nc(ROWS)
    res = run_bass_kernel_spmd(nc, in_maps, core_ids=list(range(N_CORES)))
    return _gather_output(res, orig_shape)
